# revision 15
# baseline (speedup 1.0000x reference)
"""Trainium2 Bass kernel for nn_Attention_6932077215914 (GQA attention layer).

Strategy (8 NeuronCores, tensor-parallel over heads + sequence-parallel dense):
  - Host prep: x -> x^T (bf16), softmax scale folded into w_q, w_q/w_kv sharded
    by head/KV-group, weights pre-tiled to SBUF layout. bf16 compute, fp32 PSUM.
  - Core c owns heads {2c, 2c+1} (KV group c//2). Within each core pair the KV
    projection is split (even cores K^T, odd cores V^T; split lives in the
    DATA so the graph stays SPMD-uniform); a per-batch 2-rank AllGather
    exchanges the halves under the remaining Q-projection matmuls.
  - v2 restructure (from perfetto analysis of the 367us baseline):
    * x^T loaded in 512-col chunks (sc-major) so the first projection matmuls
      start at ~5us instead of 18us.
    * Tiny dummy collectives at t=0 absorb the ~22us cold-start of the CC
      path, so AG(b=0) completes while phase 1 is still running.
    * Post-AG K/V reads issue before any later collective triggers (XBAR
      transposes serialize against in-flight collectives).
    * Phase 2 per chunk is software-pipelined: scores(qg+1) emitted before
      PV(qg) so ScalarE exp / GpSimd affine_select stay ahead of the PE;
      affine_select narrowed to the 128-col diagonal block (the rest of the
      row is never masked); no memsets (prefix garbage is never read).
    * Row sums: bf16 partial-add chains split across DVE and GpSimd, folded
      on PE with bf16 ones (f32 moving operands stream at half rate);
      reciprocal on the [1,512] row, broadcast of the inverse by a rank-1
      matmul, ctx normalized by one DVE multiply.
  - AllToAll per (batch, local-head) chunk redistributes ctx^T so each core
    owns a 256-token slice for the dense projection (gathered ctx^T chunks
    stationary, w_dense moving, final [t, o] rows written directly).
"""

import sys
import types

import numpy as np
import ml_dtypes

B, SQ, HIDDEN = 2, 2048, 2048
HEADS, GROUPS, KVC = 16, 4, 128
SCALE = KVC ** -0.5
NCORES = 8
T = B * SQ               # 4096 flattened tokens
TC = 512                 # t-chunk for QKV projection
NTC = T // TC            # 8


def _install_ntff_hook():
    """boot() skips NTFF hook registration when the image's antenv lacks
    axon_hooks; recreate the tiny module so trace=True / BASS_TRACE works."""
    if "antenv.axon_hooks" in sys.modules:
        return
    try:
        from trn_agent_boot.trn_boot import _ntff_profile_via_ctypes
        hook = _ntff_profile_via_ctypes("/opt/axon/libaxon_pjrt.so")
    except Exception:
        return
    mod = types.ModuleType("antenv.axon_hooks")
    mod.get_axon_ntff_profile_hook = lambda: hook
    mod.set_axon_ntff_profile_hook = lambda h: None
    sys.modules["antenv.axon_hooks"] = mod


_install_ntff_hook()

_CACHE = {}


def _build():
    import concourse.bass as bass
    import concourse.mybir as mybir
    import concourse.tile as tile
    from concourse import bacc
    from concourse.bass import ts, ds

    BF16 = mybir.dt.bfloat16
    F32 = mybir.dt.float32
    AF = mybir.ActivationFunctionType

    nc = bacc.Bacc("TRN2", target_bir_lowering=False, debug=False,
                   num_devices=NCORES)

    xt = nc.dram_tensor("xt", [HIDDEN, T], BF16, kind="ExternalInput")
    wq = nc.dram_tensor("wq", [128, 16 * 256], BF16, kind="ExternalInput")
    # per-core HALF of the kv projection: even cores w_k, odd cores w_v
    wkv = nc.dram_tensor("wkv", [128, 16 * 128], BF16, kind="ExternalInput")
    wd = nc.dram_tensor("wd", [128, 16 * HIDDEN], BF16, kind="ExternalInput")
    out = nc.dram_tensor("out", [512, HIDDEN], F32, kind="ExternalOutput")

    def off_of(kt, qg):
        r = kt - 4 * qg
        return 128 * r if r > 0 else 0

    with tile.TileContext(nc) as tc:
        import contextlib
        with contextlib.ExitStack() as ctx:
            persist = ctx.enter_context(tc.tile_pool(name="persist", bufs=1))
            dram = ctx.enter_context(tc.tile_pool(name="dram", bufs=1, space="DRAM"))

            ones_col_bf = persist.tile([128, 1], BF16, name="ones_col_bf")
            nc.vector.memset(ones_col_bf[:], 1.0)
            ones_row_bf = persist.tile([1, 128], BF16, name="ones_row_bf")
            nc.vector.memset(ones_row_bf[:], 1.0)

            # tiny exp at t=0 preloads the ACT function table
            dum_exp = persist.tile([1, 16], F32, name="dum_exp")
            nc.vector.memset(dum_exp[:], 0.0)
            nc.scalar.activation(dum_exp[:], dum_exp[:], AF.Exp)

            # resident Q^T / K^T / V per batch (bf16)
            q_res = [[persist.tile([128, SQ], BF16, name=f"q{h}{b}")
                      for b in range(B)] for h in range(2)]
            k_res = [persist.tile([128, SQ], BF16, name=f"k{b}") for b in range(B)]
            v_res = [persist.tile([128, 16, 128], BF16, name=f"v{b}") for b in range(B)]

            # A2A bounce buffers, one per (b, h_local) chunk
            cc_in = [[dram.tile([NCORES * 128, 256], BF16, name=f"ccin{b}{h}")
                      for h in range(2)] for b in range(B)]
            cc_out = [[dram.tile([NCORES, 128, 256], BF16, name=f"ccout{b}{h}")
                       for h in range(2)] for b in range(B)]
            # pair K/V exchange bounce buffers, per batch
            kv_in = [dram.tile([128, SQ], BF16, name=f"kvin{b}") for b in range(B)]
            kv_out = [dram.tile([2, 128, SQ], BF16, name=f"kvout{b}")
                      for b in range(B)]

            # ---- phase 1: QKV projections ----
            with tc.tile_pool(name="p1w", bufs=1) as p1w, \
                 tc.tile_pool(name="p1", bufs=2) as p1, \
                 tc.tile_pool(name="p1x", bufs=1) as p1x, \
                 tc.tile_pool(name="p1ps", bufs=2, space="PSUM") as p1ps:
                wq_sb = p1w.tile([128, 16, 256], BF16, name="wq_sb")
                nc.sync.dma_start(out=wq_sb[:], in_=wq.ap())
                wkv_sb = p1w.tile([128, 16, 128], BF16, name="wkv_sb")
                nc.sync.dma_start(out=wkv_sb[:], in_=wkv.ap())

                # x^T loaded in 512-col chunks, sc-major, matching the tci
                # consumption order so tci=0 matmuls start after ~2MB; issued
                # from the (otherwise idle) Scalar queue so the 128 triggers
                # don't back-pressure the Sync queue's kv_in writes
                xt_sb = [p1x.tile([128, T], BF16, name=f"xt{hc}")
                         for hc in range(16)]
                for sc in range(NTC):
                    for hc in range(16):
                        nc.scalar.dma_start(out=xt_sb[hc][:, ts(sc, TC)],
                                            in_=xt.ap()[ts(hc, 128), ts(sc, TC)])

                for tci in range(NTC):
                    b, sc = tci // 4, tci % 4
                    outs = []
                    for name, nbufs, w_ap in (("kv", 2, wkv_sb[:, :, :]),
                                              ("q0", 3, wq_sb[:, :, 0:128]),
                                              ("q1", 3, wq_sb[:, :, 128:256])):
                        ps = p1ps.tile([128, TC], F32, tag=name, bufs=nbufs)
                        for hc in range(16):
                            nc.tensor.matmul(ps[:], w_ap[:, hc, :],
                                             xt_sb[hc][:, ts(tci, TC)],
                                             start=(hc == 0), stop=(hc == 15))
                        outs.append(ps)
                    kvt_sb = p1.tile([128, TC], BF16, tag="kvt")
                    nc.vector.tensor_copy(kvt_sb[:], outs[0][:])
                    nc.sync.dma_start(out=kv_in[b][:, ts(sc, TC)],
                                      in_=kvt_sb[:])
                    nc.vector.tensor_copy(q_res[0][b][:, ts(sc, TC)], outs[1][:])
                    nc.vector.tensor_copy(q_res[1][b][:, ts(sc, TC)], outs[2][:])
                    if sc == 3:
                        # exchange K/V within the core pair (pair-rank 0 = K)
                        nc.gpsimd.collective_compute(
                            "AllGather", mybir.AluOpType.bypass,
                            replica_groups=[[2 * i, 2 * i + 1]
                                            for i in range(NCORES // 2)],
                            ins=[kv_in[b].opt()],
                            outs=[kv_out[b].opt()])
                        for kc in range(4):
                            nc.sync.dma_start(out=k_res[b][:, ts(kc, TC)],
                                              in_=kv_out[b][0, :, ts(kc, TC)])
                        for s4 in range(4):
                            nc.sync.dma_start(
                                out=v_res[b][:, ds(4 * s4, 4), :],
                                in_=kv_out[b][1, :, ts(s4, TC)],
                                transpose=True)

            # dense weights + A2A gather tiles reuse the freed x^T SBUF;
            # loads overlap attention
            wdp = ctx.enter_context(tc.tile_pool(name="wdp", bufs=1))
            wd_sb = wdp.tile([128, 16, HIDDEN], BF16, name="wd_sb")
            nc.sync.dma_start(out=wd_sb[:], in_=wd.ap())
            p3g = ctx.enter_context(tc.tile_pool(name="p3g", bufs=1))
            g_all = [[p3g.tile([128, NCORES, 256], BF16, name=f"g{b}{h}")
                      for h in range(2)] for b in range(B)]

            # ---- phase 2: attention per (b, h_local), scores^T [k, q] ----
            # ScalarE's exp is the phase-2 bottleneck engine (~22us/chunk),
            # and with 4 score-PSUM bufs the PE score stream is gated on exp
            # draining them, so (a) full k-tile PAIRS share one [128,2,512]
            # score PSUM + one merged exp (fewer ACT fixed overheads), and
            # (b) PV/fold/broadcast matmuls are zipped between score matmuls,
            # with each chunk's qg2/qg3 tail deferred into the NEXT chunk's
            # score stream.
            CHUNKS = [(0, 0), (0, 1), (1, 0), (1, 1)]
            # full-width (off=0) tiles summed on GpSimd; diagonal + head on DVE
            GPS_SPLIT = {0: [], 1: [2, 3], 2: [2, 3, 4, 5, 6, 7],
                         3: [4, 5, 6, 7, 8, 9]}
            with tc.tile_pool(name="p2", bufs=1) as p2, \
                 tc.tile_pool(name="p2c", bufs=2) as p2c, \
                 tc.tile_pool(name="p2s", bufs=2) as p2s, \
                 tc.tile_pool(name="p2ps", bufs=2, space="PSUM") as p2ps, \
                 tc.tile_pool(name="p2sc", bufs=4, space="PSUM") as p2sc:
                prev_core, prev_bc3 = [], None
                for ci, (b, hl) in enumerate(CHUNKS):
                    q_src = q_res[hl][b]
                    st = {}
                    for qg in range(4):
                        st[("et", qg)] = p2.tile([128, 4 * (qg + 1), 512],
                                                 BF16, tag=f"et{qg}",
                                                 name=f"et{qg}")

                    def emit_sc_one(qg, kt, st=st, b=b, q_src=q_src):
                        slab = st[("et", qg)]
                        o = off_of(kt, qg)
                        w = 512 - o
                        sc_ps = p2sc.tile([128, 512], F32, tag="sc",
                                          name="scs")
                        nc.tensor.matmul(
                            sc_ps[:, 0:w], k_res[b][:, ts(kt, 128)],
                            q_src[:, ds(qg * 512 + o, w)],
                            start=True, stop=True)
                        nc.scalar.activation(
                            slab[:, kt, o:512], sc_ps[:, 0:w], AF.Exp)
                        if kt - 4 * qg >= 0:
                            # zero E^T where q < k inside the 128-wide
                            # diagonal block (beyond it q >= k always)
                            nc.gpsimd.affine_select(
                                out=slab[:, kt, o:o + 128],
                                in_=slab[:, kt, o:o + 128],
                                compare_op=mybir.AluOpType.is_ge,
                                fill=0.0, base=0,
                                pattern=[[1, 128]],
                                channel_multiplier=-1)

                    def emit_chain(qg, st=st):
                        nkt = 4 * (qg + 1)
                        slab = st[("et", qg)]
                        gps = GPS_SPLIT[qg]
                        dve = [kt for kt in range(nkt) if kt not in gps]
                        padd = p2s.tile([128, 512], BF16, tag=f"padd{qg}",
                                        name=f"padd{qg}")
                        if qg == 0:
                            nc.vector.tensor_copy(padd[:], slab[:, 0, :])
                            rest = dve[1:]
                        else:
                            nc.vector.tensor_add(padd[:], slab[:, dve[0], :],
                                                 slab[:, dve[1], :])
                            rest = dve[2:]
                        for kt in rest:
                            o = off_of(kt, qg)
                            nc.vector.tensor_add(padd[:, o:512], padd[:, o:512],
                                                 slab[:, kt, o:512])
                        padd_g = None
                        if gps:
                            padd_g = p2s.tile([128, 512], BF16,
                                              tag=f"paddg{qg}",
                                              name=f"paddg{qg}")
                            nc.gpsimd.tensor_add(padd_g[:], slab[:, gps[0], :],
                                                 slab[:, gps[1], :])
                            for kt in gps[2:]:
                                nc.gpsimd.tensor_add(padd_g[:], padd_g[:],
                                                     slab[:, kt, :])
                        st[("padd", qg)] = (padd, padd_g)

                    def make_pv(qg, kt, st=st, b=b):
                        nkt = 4 * (qg + 1)

                        def thunk():
                            if kt == 0:
                                st[("ctx", qg)] = p2ps.tile(
                                    [128, 512], F32, tag="ctx", bufs=3,
                                    name=f"ctx{qg}")
                            o = off_of(kt, qg)
                            nc.tensor.matmul(
                                st[("ctx", qg)][:, o:512],
                                v_res[b][:, kt, :],
                                st[("et", qg)][:, kt, o:512],
                                start=(kt == 0), stop=(kt == nkt - 1),
                                skip_group_check=True)
                        return thunk

                    def make_fold(qg, st=st):
                        def thunk():
                            padd, padd_g = st[("padd", qg)]
                            bc_ps = p2ps.tile([128, 512], F32, tag="bc",
                                              bufs=1, name="bc")
                            st[("bcps", qg)] = bc_ps
                            # row sums land in partition row 0 of the bc
                            # bank; the broadcast matmul overwrites the bank
                            # after the row is copied out
                            nc.tensor.matmul(bc_ps[0:1, :], ones_col_bf[:],
                                             padd[:],
                                             start=True, stop=(padd_g is None),
                                             skip_group_check=True)
                            if padd_g is not None:
                                nc.tensor.matmul(bc_ps[0:1, :], ones_col_bf[:],
                                                 padd_g[:], start=False,
                                                 stop=True,
                                                 skip_group_check=True)
                            rs_bf = p2s.tile([1, 512], BF16, tag="rsbf",
                                             name="rsbf")
                            nc.vector.tensor_copy(rs_bf[:], bc_ps[0:1, :])
                            st[("rsbf", qg)] = rs_bf
                        return thunk

                    def make_bc(qg, st=st, b=b, hl=hl, last=False):
                        def thunk():
                            bc_ps = st[("bcps", qg)]
                            nc.tensor.matmul(bc_ps[:], ones_row_bf[:],
                                             st[("rsbf", qg)][:],
                                             start=True, stop=True)
                            rinv = p2s.tile([128, 512], F32, tag="rinv",
                                            name="rinv")
                            nc.vector.reciprocal_approx_fast(rinv[:], bc_ps[:])
                            ctxt = p2c.tile([128, 512], BF16, tag="ctxt",
                                            name="ctxt")
                            nc.vector.tensor_mul(ctxt[:], st[("ctx", qg)][:],
                                                 rinv[:])
                            for half in range(2):
                                peer = 2 * qg + half
                                nc.sync.dma_start(
                                    out=cc_in[b][hl][ts(peer, 128), :],
                                    in_=ctxt[:, ts(half, 256)])
                            if last:
                                nc.gpsimd.collective_compute(
                                    "AllToAll", mybir.AluOpType.bypass,
                                    replica_groups=[list(range(NCORES))],
                                    ins=[cc_in[b][hl].opt()],
                                    outs=[cc_out[b][hl].opt()])
                                nc.sync.dma_start(
                                    out=g_all[b][hl][:],
                                    in_=cc_out[b][hl].rearrange(
                                        "i p s -> p i s"))
                        return thunk

                    P = {qg: [make_pv(qg, kt) for kt in range(4 * (qg + 1))]
                         for qg in range(4)}
                    F = {qg: make_fold(qg) for qg in range(4)}
                    BC = {qg: make_bc(qg, last=(qg == 3)) for qg in range(4)}

                    # fill order staggers folds/broadcasts away from their
                    # DVE dependencies (fold after its chain, bc ~recip-lag
                    # after its fold, muls between consecutive bc's)
                    fills = list(prev_core) + list(P[0])
                    if prev_bc3 is not None:
                        fills.append(prev_bc3)
                    fills += [F[0], P[1][0], P[1][1], BC[0], *P[1][2:],
                              F[1], BC[1]]
                    prev_core = [*P[2], F[2], P[3][0], P[3][1], BC[2],
                                 *P[3][2:], F[3]]
                    prev_bc3 = BC[3]

                    gate = 12 if ci == 0 else 0
                    fi = 0
                    si = 0
                    for qg in range(4):
                        nkt = 4 * (qg + 1)
                        for kt in range(nkt):
                            emit_sc_one(qg, kt)
                            pop_ok = (si >= gate if ci > 0 else
                                      (si >= gate and (si - gate) % 2 == 0))
                            if pop_ok and fi < len(fills):
                                fills[fi]()
                                fi += 1
                            si += 1
                        emit_chain(qg)
                    while fi < len(fills):
                        fills[fi]()
                        fi += 1
                # tail of the last chunk
                for t in prev_core:
                    t()
                prev_bc3()

            # ---- phase 3: dense projection on my 256-token slice per batch ----
            with tc.tile_pool(name="p3", bufs=2) as p3, \
                 tc.tile_pool(name="p3ps", bufs=2, space="PSUM") as p3ps:
                for b in range(B):
                    for u in range(2):
                        o_ps = p3ps.tile([128, HIDDEN], F32, tag="ops")
                        o_sb = p3.tile([128, HIDDEN], F32, tag="osb")
                        for oc in range(4):
                            # hl=0 block first: its A2A lands one chunk
                            # earlier than hl=1's
                            for ec in range(16):
                                hl, i = ec // 8, ec % 8
                                nc.tensor.matmul(
                                    o_ps[:, ts(oc, 512)],
                                    g_all[b][hl][:, i, ts(u, 128)],
                                    wd_sb[:, 2 * i + hl, ts(oc, 512)],
                                    start=(ec == 0), stop=(ec == 15))
                            nc.vector.tensor_copy(o_sb[:, ts(oc, 512)],
                                                  o_ps[:, ts(oc, 512)])
                            nc.sync.dma_start(
                                out=out.ap()[ds(b * 256 + u * 128, 128),
                                             ts(oc, 512)],
                                in_=o_sb[:, ts(oc, 512)])

    nc.compile()
    return nc


def kernel(x, w_q, w_kv, w_dense):
    from concourse.bass_utils import run_bass_kernel_spmd

    bf16 = ml_dtypes.bfloat16
    x = np.asarray(x, dtype=np.float32)
    w_q = np.asarray(w_q, dtype=np.float32)
    w_kv = np.asarray(w_kv, dtype=np.float32)
    w_dense = np.asarray(w_dense, dtype=np.float32)

    xt = np.ascontiguousarray(x.reshape(T, HIDDEN).T).astype(bf16)
    wq_s = (w_q * SCALE).astype(bf16)          # fold softmax scale into Q proj
    wkv_b = w_kv.astype(bf16)
    wd_b = w_dense.astype(bf16)

    def pretile(w):
        # [2048, e] -> SBUF layout [p, hc*e]: row p, col hc*e_sz + e
        e_sz = w.shape[1]
        return np.ascontiguousarray(
            w.reshape(16, 128, e_sz).transpose(1, 0, 2).reshape(128, 16 * e_sz))

    wd_t = pretile(wd_b)
    in_maps = []
    for c in range(NCORES):
        g = c // 2
        if c % 2 == 0:
            wkv_c = wkv_b[:, 128 * g:128 * (g + 1)]                # K half
        else:
            wkv_c = wkv_b[:, 512 + 128 * g:512 + 128 * (g + 1)]    # V half
        in_maps.append({
            "xt": xt,
            "wq": pretile(wq_s[:, 256 * c:256 * (c + 1)]),
            "wkv": pretile(wkv_c),
            "wd": wd_t,
        })

    if "nc" not in _CACHE:
        _CACHE["nc"] = _build()
    nc = _CACHE["nc"]

    res = run_bass_kernel_spmd(nc, in_maps, core_ids=list(range(NCORES)))
    kernel.last_results = res
    kernel.last_exec_time_ns = res.exec_time_ns

    out_full = np.empty((T, HIDDEN), dtype=np.float32)
    for c in range(NCORES):
        r = res.results[c]["out"]              # [512, 2048]
        for b in range(B):
            out_full[b * SQ + 256 * c: b * SQ + 256 * (c + 1), :] = \
                r[b * 256:(b + 1) * 256, :]
    return out_full.reshape(B, SQ, HIDDEN)
